# revision 4
# baseline (speedup 1.0000x reference)
"""Multi-head causal self-attention on 8 Trainium2 NeuronCores (Bass/Tile).

Problem: y = proj(softmax(causal_mask(Q K^T / sqrt(D))) V) for B=2, T=2048,
C=1024, H=16 heads, D=64.

Sharding (tensor-parallel over heads, 8-way):
  - Core i owns heads {2i, 2i+1}. It computes qT/kT/vT for its heads over
    both batches (full x, its 128-column slice of Wqkv), runs causal
    attention per head fully on-core, producing normalized yT_local
    [128, B*T] (head-dims on partitions, time on the free axis).
  - One 8-way AllToAll reshards head-split -> time-split: core j ends with
    yT_full [1024, 512] for time-slice j of the flattened (B*T) axis and
    computes its [512, 1024] slice of y @ Wproj.
  - The host concatenates the 8 time-slices into [2, 2048, 1024].

Matmuls use the float32r PE path (fp32 rounded to 11 explicit mantissa
bits; 1 column/cycle when the moving operand is >= 256 wide — 4x the
plain-fp32 rate; accumulation stays full fp32 in PSUM). The BIR verifier
requires fp32r matmul operands to carry the float32r dtype from their
producer, so host-prepared tensors are pre-rounded on the host and DMA'd
into float32r tiles, and on-chip producers (PSUM-evacuation copies, exp)
write float32r directly.

Causality is exact and exploited: S^T blocks strictly above the diagonal
are skipped; diagonal-band blocks use a restricted column range plus a
triangular multiplicative mask after exp.

Softmax layout trick: scores are computed transposed (S^T[k, q], keys on
partitions) so no transposes are needed anywhere in the attention path.
exp goes on ScalarE; the denominator comes free from a ones column
appended to V (row 64 of the P@V accumulator); 1/denominator is broadcast
across partitions with a K=1 PE matmul against a ones row-vector.
"""

import numpy as np

import concourse.bass as bass
import concourse.mybir as mybir
import concourse.tile as tile
from concourse import bacc
from concourse import bass_utils

F32 = mybir.dt.float32
F32R = mybir.dt.float32r
AF = mybir.ActivationFunctionType

B, T, C = 2, 2048, 1024
H, D = 16, 64
N_CORES = 8
HL = H // N_CORES        # heads per core = 2
NCT = C // 128           # contraction tiles = 8
NQ = T // 512            # q tiles per batch = 4
NK = T // 128            # k tiles per batch = 16
SCALE = 1.0 / float(np.sqrt(D))  # 0.125

_BUILD_CACHE = {}


def round_f32r(x):
    """fp32 -> fp32r rounding (11 explicit mantissa bits, nearest-even)."""
    u = np.asarray(x, np.float32).view(np.uint32).astype(np.uint64)
    low = u & np.uint64(0xFFF)
    base = u & np.uint64(0xFFFFF000)
    lsb = (u >> np.uint64(12)) & np.uint64(1)
    round_up = (low > 0x800) | ((low == 0x800) & (lsb == 1))
    out = base + np.where(round_up, np.uint64(0x1000), np.uint64(0))
    return (out & np.uint64(0xFFFFFFFF)).astype(np.uint32).view(np.float32)


def build_kernel(apply_pad_mask: bool):
    nc = bacc.Bacc(
        "TRN2", target_bir_lowering=False, debug=False, num_devices=N_CORES
    )
    # f32r inputs are pre-rounded host-side; DMA'd straight into f32r tiles
    xT = nc.dram_tensor("xT", [C, B * T], F32R, kind="ExternalInput").ap()
    wqkv = nc.dram_tensor("wqkv", [C, 3 * HL * D], F32R, kind="ExternalInput").ap()
    wo = nc.dram_tensor("wo", [C, C], F32R, kind="ExternalInput").ap()
    tri = nc.dram_tensor("tri", [128, 128], F32, kind="ExternalInput").ap()
    ident = nc.dram_tensor("ident", [128, 128], F32R, kind="ExternalInput").ap()
    padk = nc.dram_tensor("padk", [128, B * NK], F32, kind="ExternalInput").ap()
    out = nc.dram_tensor("out", [512, C], F32, kind="ExternalOutput").ap()

    with tile.TileContext(nc) as tc:
        with (
            tc.tile_pool(name="const", bufs=1) as constp,
            tc.tile_pool(name="qk", bufs=2) as qkp,
            tc.tile_pool(name="vv", bufs=1) as vvp,
            tc.tile_pool(name="work", bufs=2) as wk,
            tc.tile_pool(name="ytmp_pool", bufs=4) as ytp,
            tc.tile_pool(name="ps_main", bufs=3, space="PSUM") as ps_main,
            tc.tile_pool(name="ps_s", bufs=3, space="PSUM") as ps_s,
            tc.tile_pool(name="ps_y", bufs=2, space="PSUM") as ps_y,
            tc.tile_pool(name="dram", bufs=1, space="DRAM") as dram,
        ):
            # ---------------- constants ----------------
            tri_sb = constp.tile([128, 128], F32, name="tri_sb")
            nc.sync.dma_start(tri_sb[:], tri[:])
            id_sb = constp.tile([128, 128], F32R, name="id_sb")
            nc.sync.dma_start(id_sb[:], ident[:])
            # ones row on partition 64 (base-partition-matched with the
            # psum_y sums row for the K=1 broadcast matmul)
            ones_f = constp.tile([65, 64], F32, name="ones_f")
            nc.vector.memset(ones_f[64:65, :], 1.0)
            ones65 = constp.tile([65, 64], F32R, name="ones65")
            nc.gpsimd.tensor_copy(ones65[64:65, :], ones_f[64:65, :])
            # ones for the V ones-columns
            onesc_f = constp.tile([128, HL], F32, name="onesc_f")
            nc.vector.memset(onesc_f[:], 1.0)
            onesc = constp.tile([128, HL], F32R, name="onesc")
            nc.gpsimd.tensor_copy(onesc[:], onesc_f[:])
            if apply_pad_mask:
                padk_sb = constp.tile([128, B * NK], F32, name="padk_sb")
                nc.sync.dma_start(padk_sb[:], padk[:])

            a2a_in = dram.tile([N_CORES, 128, 512], F32R, name="a2a_in")
            a2a_out = dram.tile([N_CORES, 128, 512], F32R, name="a2a_out")

            with (
                tc.tile_pool(name="x_pool", bufs=1) as xp,
                tc.tile_pool(name="wq_pool", bufs=1) as wqp,
            ):
                # ------------ weights (once) ------------
                wqkv_sb = []
                for ct in range(NCT):
                    w_sb = wqp.tile([128, 3 * HL * D], F32R,
                                    name=f"wqkv{ct}", tag=f"wqkv{ct}")
                    nc.sync.dma_start(
                        w_sb[:], wqkv[ct * 128:(ct + 1) * 128, :]
                    )
                    wqkv_sb.append(w_sb)

                for b in range(B):
                    # ------------ xT for batch b ------------
                    xt_sb = []
                    for ct in range(NCT):
                        x_sb = xp.tile([128, T], F32R, name=f"xt{ct}",
                                       tag=f"xt{ct}")
                        nc.sync.dma_start(
                            x_sb[:],
                            xT[ct * 128:(ct + 1) * 128, b * T:(b + 1) * T],
                        )
                        xt_sb.append(x_sb)

                    # ------------ qT / kT / vT projections ------------
                    qT = qkp.tile([128, T], F32R, name="qT", tag="qT")
                    kT = qkp.tile([128, T], F32R, name="kT", tag="kT")
                    vT = qkp.tile([128, T], F32R, name="vT", tag="vT")
                    for which, dst in ((0, qT), (1, kT), (2, vT)):
                        for n in range(NQ):
                            p = ps_main.tile([128, 512], F32, name="p_mm",
                                             tag="ps")
                            for ct in range(NCT):
                                nc.tensor.matmul(
                                    p[:],
                                    wqkv_sb[ct][:, which * 128:(which + 1) * 128],
                                    xt_sb[ct][:, n * 512:(n + 1) * 512],
                                    start=(ct == 0),
                                    stop=(ct == NCT - 1),
                                )
                            nc.vector.tensor_copy(
                                dst[:, n * 512:(n + 1) * 512], p[:]
                            )

                    # ------------ V: transpose vT into [keys, dims] -------
                    V = []
                    for kt in range(NK):
                        v_sb = vvp.tile([128, HL * 65], F32R, name=f"V{kt}",
                                        tag=f"V{kt}")
                        pt = ps_main.tile([128, 128], F32R, name="p_tr",
                                          tag="ps")
                        nc.tensor.transpose(
                            pt[:], vT[:, kt * 128:(kt + 1) * 128], id_sb[:]
                        )
                        v3 = v_sb[:].rearrange("p (h e) -> p h e", h=HL)
                        nc.gpsimd.tensor_copy(v3[:, :, 64], onesc[:])
                        nc.vector.tensor_copy(
                            v3[:, :, 0:64],
                            pt[:].rearrange("p (h e) -> p h e", h=HL),
                        )
                        V.append(v_sb)

                    # ------------ attention ------------
                    for h in range(HL):
                        h0 = h * 64
                        for j in range(NQ):
                            q0 = j * 512
                            py = ps_y.tile([65, 512], F32, name="p_y",
                                           tag="py")
                            n_kt = 4 * j + 4
                            for kt in range(n_kt):
                                i = kt - 4 * j  # diagonal index if >= 0
                                off = 128 * i if i >= 0 else 0
                                ps = ps_s.tile([128, 512], F32, name="p_s",
                                               tag="pss")
                                nc.tensor.matmul(
                                    ps[:, off:512],
                                    kT[h0:h0 + 64, kt * 128:(kt + 1) * 128],
                                    qT[h0:h0 + 64, q0 + off:q0 + 512],
                                    start=True,
                                    stop=True,
                                )
                                p_sb = wk.tile([128, 512], F32R, name="p_sb",
                                               tag="p_sb", bufs=4)
                                nc.scalar.activation(
                                    p_sb[:, off:512], ps[:, off:512], AF.Exp,
                                    scale=float(SCALE),
                                )
                                if i >= 0:
                                    nc.vector.tensor_mul(
                                        p_sb[:, off:off + 128],
                                        p_sb[:, off:off + 128],
                                        tri_sb[:],
                                    )
                                if apply_pad_mask:
                                    nc.vector.tensor_scalar_mul(
                                        p_sb[:, off:512],
                                        p_sb[:, off:512],
                                        padk_sb[:, b * NK + kt:
                                                b * NK + kt + 1],
                                    )
                                nc.tensor.matmul(
                                    py[0:65, off:512],
                                    V[kt][:, h * 65:(h + 1) * 65],
                                    p_sb[:, off:512],
                                    start=(kt == 0),
                                    stop=(kt == n_kt - 1),
                                )
                            # normalize: yTn = py[0:64] * bcast(1/py[64])
                            sums = wk.tile([65, 512], F32, name="sums",
                                           tag="sums")
                            nc.vector.tensor_copy(sums[64:65, :],
                                                  py[64:65, :])
                            rec = wk.tile([65, 512], F32R, name="rec",
                                          tag="rec")
                            with nc.allow_low_precision(
                                    reason="fp32r softmax denom"):
                                nc.vector.reciprocal(rec[64:65, :],
                                                     sums[64:65, :])
                            pb = ps_s.tile([64, 512], F32, name="p_b",
                                           tag="pss")
                            nc.tensor.matmul(
                                pb[:],
                                ones65[64:65, :],
                                rec[64:65, :],
                                start=True,
                                stop=True,
                            )
                            yt = wk.tile([64, 512], F32, name="yt", tag="yt")
                            nc.vector.tensor_copy(yt[:], py[0:64, :])
                            ytn = ytp.tile([64, 512], F32R, name="ytn",
                                           tag="ytn")
                            nc.vector.tensor_mul(ytn[:], yt[:], pb[:])
                            # stage into the AllToAll send buffer: shard
                            # d = b*4 + j, rows h*64 .. h*64+63
                            d = b * NQ + j
                            nc.sync.dma_start(a2a_in[d, h0:h0 + 64, :],
                                              ytn[:])

            # ---------------- AllToAll (head-split -> time-split) ---------
            nc.gpsimd.collective_compute(
                "AllToAll",
                mybir.AluOpType.bypass,
                replica_groups=[list(range(N_CORES))],
                ins=[a2a_in.opt()],
                outs=[a2a_out.opt()],
            )

            # ---------------- output projection ----------------
            with (
                tc.tile_pool(name="wo_pool", bufs=1) as wop,
                tc.tile_pool(name="ytf_pool", bufs=1) as yfp,
            ):
                wo_sb = []
                for ct in range(NCT):
                    w_sb = wop.tile([128, C], F32R, name=f"wo{ct}",
                                    tag=f"wo{ct}")
                    nc.sync.dma_start(w_sb[:], wo[ct * 128:(ct + 1) * 128, :])
                    wo_sb.append(w_sb)
                ytf = []
                for s in range(N_CORES):
                    y_sb = yfp.tile([128, 512], F32R, name=f"ytf{s}",
                                    tag=f"ytf{s}")
                    nc.sync.dma_start(y_sb[:], a2a_out[s, :, :])
                    ytf.append(y_sb)
                for mt in range(4):
                    o_sb = wk.tile([128, C], F32, name="o_sb", tag="o_sb")
                    for n in range(2):
                        po = ps_main.tile([128, 512], F32, name="p_o",
                                          tag="ps")
                        for ct in range(NCT):
                            nc.tensor.matmul(
                                po[:],
                                ytf[ct][:, mt * 128:(mt + 1) * 128],
                                wo_sb[ct][:, n * 512:(n + 1) * 512],
                                start=(ct == 0),
                                stop=(ct == NCT - 1),
                            )
                        nc.vector.tensor_copy(
                            o_sb[:, n * 512:(n + 1) * 512], po[:]
                        )
                    nc.sync.dma_start(out[mt * 128:(mt + 1) * 128, :],
                                      o_sb[:])

    nc.compile()
    return nc


def _host_inputs(x, tok_mask, Wqkv, Wproj, apply_pad_mask):
    x = np.ascontiguousarray(np.asarray(x, dtype=np.float32))
    Wqkv = np.ascontiguousarray(np.asarray(Wqkv, dtype=np.float32))
    Wproj = np.ascontiguousarray(np.asarray(Wproj, dtype=np.float32))
    # xT: [C, B*T], time-major = [b0 t0..T-1 | b1 t0..T-1], pre-rounded
    xT = round_f32r(np.concatenate([x[b].T for b in range(B)], axis=1))
    wo_r = round_f32r(Wproj)
    r = np.arange(128)
    tri = (r[None, :] >= r[:, None]).astype(np.float32)  # keep if col >= row
    ident = np.eye(128, dtype=np.float32)
    if apply_pad_mask:
        padk = np.zeros((128, B * NK), np.float32)
        for b in range(B):
            padk[:, b * NK:(b + 1) * NK] = (
                np.asarray(tok_mask[b]).reshape(NK, 128).T.astype(np.float32)
            )
    else:
        padk = np.ones((128, B * NK), np.float32)

    in_maps = []
    for core in range(N_CORES):
        cols = slice(core * HL * D, (core + 1) * HL * D)  # 128 columns
        wqkv_c = round_f32r(
            np.concatenate(
                [Wqkv[:, :C][:, cols], Wqkv[:, C:2 * C][:, cols],
                 Wqkv[:, 2 * C:][:, cols]],
                axis=1,
            )
        )
        in_maps.append(
            {
                "xT": xT,
                "wqkv": wqkv_c,
                "wo": wo_r,
                "tri": tri,
                "ident": ident,
                "padk": padk,
            }
        )
    return in_maps


def kernel(x, tok_mask, Wqkv, Wproj, _run_kwargs=None):
    tok = np.asarray(tok_mask)
    apply_pad_mask = not bool(tok.all())
    key = apply_pad_mask
    if key not in _BUILD_CACHE:
        _BUILD_CACHE[key] = build_kernel(apply_pad_mask)
    nc = _BUILD_CACHE[key]
    in_maps = _host_inputs(x, tok_mask, Wqkv, Wproj, apply_pad_mask)
    kw = dict(_run_kwargs or {})
    res = bass_utils.run_bass_kernel_spmd(
        nc, in_maps, core_ids=list(range(N_CORES)), **kw
    )
    out = np.empty((B, T, C), np.float32)
    for core in range(N_CORES):
        b, jj = divmod(core, NQ)
        out[b, jj * 512:(jj + 1) * 512, :] = res.results[core]["out"]
    kernel.last_result = res
    return out


# revision 8
# speedup vs baseline: 1.1751x; 1.1751x over previous
"""Multi-head causal self-attention on 8 Trainium2 NeuronCores (Bass/Tile).

Problem: y = proj(softmax(causal_mask(Q K^T / sqrt(D))) V) for B=2, T=2048,
C=1024, H=16 heads, D=64.

Sharding (tensor-parallel over heads, 8-way):
  - Core i owns heads {2i, 2i+1}: computes qT/kT/vT for its heads over both
    batches (full x, its 128-column slice of Wqkv) and runs causal attention
    per head fully on-core, producing normalized yT_local (head-dims on
    partitions, time on the free axis).
  - Two 8-way AllToAlls (one per local head-row, so the first overlaps the
    second head's compute) reshard head-split -> time-split: core j ends
    with yT_full [1024, 512] for time-slice j of the flattened (B*T) axis
    and computes its [512, 1024] slice of y @ Wproj.
  - The host concatenates the 8 time-slices into [2, 2048, 1024].

Matmuls use the float32r PE path (fp32 rounded to 11 explicit mantissa
bits, 1 column/cycle when the moving operand is >= 256 wide; accumulation
stays fp32 in PSUM). fp32r operands must carry float32r dtype from their
producer: host tensors are pre-rounded and DMA'd into float32r tiles,
on-chip producers write float32r directly.

Causality is exact: S^T blocks strictly above the diagonal are skipped,
diagonal blocks use a restricted column range + triangular multiplicative
mask after exp. Scores are computed transposed (S^T[k, q]) so the
attention path needs no transposes; the softmax denominator comes free
from a ones column appended to V (row 64 of the P@V accumulator).

Normalization is deferred out of the attention inner loop: per head-row,
the 8 denominator rows are DMA-scattered into a [32, 128] tile, inverted
with ONE wide reciprocal (all lanes busy), DMA-gathered back to rows, and
broadcast across partitions with K=1 PE matmuls at the end of the head's
compute — keeping the multi-microsecond reciprocal latency off the PE
instruction stream.
"""

import numpy as np

import concourse.bass as bass
import concourse.mybir as mybir
import concourse.tile as tile
from concourse import bacc
from concourse import bass_utils

F32 = mybir.dt.float32
F32R = mybir.dt.float32r
AF = mybir.ActivationFunctionType

B, T, C = 2, 2048, 1024
H, D = 16, 64
N_CORES = 8
HL = H // N_CORES        # heads per core = 2
NCT = C // 128           # contraction tiles = 8
NQ = T // 512            # q tiles per batch = 4
NK = T // 128            # k tiles per batch = 16
SCALE = 1.0 / float(np.sqrt(D))  # 0.125

_BUILD_CACHE = {}


def round_f32r(x):
    """fp32 -> fp32r rounding (11 explicit mantissa bits, nearest-even)."""
    u = np.asarray(x, np.float32).view(np.uint32).astype(np.uint64)
    low = u & np.uint64(0xFFF)
    base = u & np.uint64(0xFFFFF000)
    lsb = (u >> np.uint64(12)) & np.uint64(1)
    round_up = (low > 0x800) | ((low == 0x800) & (lsb == 1))
    out = base + np.where(round_up, np.uint64(0x1000), np.uint64(0))
    return (out & np.uint64(0xFFFFFFFF)).astype(np.uint32).view(np.float32)


def build_kernel(apply_pad_mask: bool):
    nc = bacc.Bacc(
        "TRN2", target_bir_lowering=False, debug=False, num_devices=N_CORES
    )
    xT = nc.dram_tensor("xT", [C, B * T], F32R, kind="ExternalInput").ap()
    wqkv = nc.dram_tensor("wqkv", [C, 3 * HL * D], F32R, kind="ExternalInput").ap()
    wo = nc.dram_tensor("wo", [C, C], F32R, kind="ExternalInput").ap()
    tri = nc.dram_tensor("tri", [128, 128], F32, kind="ExternalInput").ap()
    ident = nc.dram_tensor("ident", [128, 128], F32R, kind="ExternalInput").ap()
    padk = nc.dram_tensor("padk", [128, B * NK], F32, kind="ExternalInput").ap()
    out = nc.dram_tensor("out", [512, C], F32, kind="ExternalOutput").ap()

    with tile.TileContext(nc) as tc:
        with (
            tc.tile_pool(name="const", bufs=1) as constp,
            tc.tile_pool(name="qk", bufs=1) as qkp,
            tc.tile_pool(name="vv", bufs=1) as vvp,
            tc.tile_pool(name="work", bufs=2) as wk,
            tc.tile_pool(name="ytmp_pool", bufs=2) as ytp,
            tc.tile_pool(name="ps_main", bufs=3, space="PSUM") as ps_main,
            tc.tile_pool(name="ps_s", bufs=3, space="PSUM") as ps_s,
            tc.tile_pool(name="ps_y", bufs=2, space="PSUM") as ps_y,
            tc.tile_pool(name="dram", bufs=1, space="DRAM") as dram,
        ):
            # ---------------- constants ----------------
            tri_sb = constp.tile([128, 128], F32, name="tri_sb")
            nc.sync.dma_start(tri_sb[:], tri[:])
            id_sb = constp.tile([128, 128], F32R, name="id_sb")
            nc.sync.dma_start(id_sb[:], ident[:])
            # ones rows at the 32-aligned partitions K=1 matmuls can source
            ones_f = constp.tile([65, 64], F32, name="ones_f")
            ones_sb = constp.tile([65, 64], F32R, name="ones_sb")
            for r in (0, 32, 64):
                nc.vector.memset(ones_f[r:r + 1, :], 1.0)
                nc.gpsimd.tensor_copy(ones_sb[r:r + 1, :], ones_f[r:r + 1, :])
            # ones for the V ones-columns
            onesc_f = constp.tile([128, HL], F32, name="onesc_f")
            nc.vector.memset(onesc_f[:], 1.0)
            onesc = constp.tile([128, HL], F32R, name="onesc")
            nc.gpsimd.tensor_copy(onesc[:], onesc_f[:])
            if apply_pad_mask:
                padk_sb = constp.tile([128, B * NK], F32, name="padk_sb")
                nc.sync.dma_start(padk_sb[:], padk[:])

            # per-head-row AllToAll buffers (shard = [64, 512])
            a2a_in = [dram.tile([N_CORES, 64, 512], F32R, name=f"a2a_in{h}")
                      for h in range(HL)]
            a2a_out = [dram.tile([N_CORES, 64, 512], F32R, name=f"a2a_out{h}")
                       for h in range(HL)]

            # ============ phase 1: projections for both batches ============
            qT = [None] * B
            kT = [None] * B
            V = [[None] * NK for _ in range(B)]
            with (
                tc.tile_pool(name="x_pool", bufs=1) as xp,
                tc.tile_pool(name="wq_pool", bufs=1) as wqp,
                tc.tile_pool(name="vt_pool", bufs=1) as vtp,
            ):
                wqkv_sb = []
                for ct in range(NCT):
                    w_sb = wqp.tile([128, 3 * HL * D], F32R,
                                    name=f"wqkv{ct}", tag=f"wqkv{ct}")
                    nc.sync.dma_start(w_sb[:], wqkv[ct * 128:(ct + 1) * 128, :])
                    wqkv_sb.append(w_sb)

                for b in range(B):
                    xt_sb = []
                    for ct in range(NCT):
                        x_sb = xp.tile([128, T], F32R, name=f"xt{ct}",
                                       tag=f"xt{ct}")
                        nc.sync.dma_start(
                            x_sb[:],
                            xT[ct * 128:(ct + 1) * 128, b * T:(b + 1) * T],
                        )
                        xt_sb.append(x_sb)

                    qT[b] = qkp.tile([128, T], F32R, name="qT", tag=f"qT{b}")
                    kT[b] = qkp.tile([128, T], F32R, name="kT", tag=f"kT{b}")
                    vT = vtp.tile([128, T], F32R, name="vT", tag="vT")
                    for which, dst in ((0, qT[b]), (1, kT[b]), (2, vT)):
                        for n in range(NQ):
                            p = ps_main.tile([128, 512], F32, name="p_mm",
                                             tag="ps")
                            for ct in range(NCT):
                                nc.tensor.matmul(
                                    p[:],
                                    wqkv_sb[ct][:, which * 128:(which + 1) * 128],
                                    xt_sb[ct][:, n * 512:(n + 1) * 512],
                                    start=(ct == 0),
                                    stop=(ct == NCT - 1),
                                )
                            nc.vector.tensor_copy(
                                dst[:, n * 512:(n + 1) * 512], p[:]
                            )

                    for kt in range(NK):
                        v_sb = vvp.tile([128, HL * 65], F32R,
                                        name=f"V{b}_{kt}", tag=f"V{b}_{kt}")
                        pt = ps_main.tile([128, 128], F32R, name="p_tr",
                                          tag="ps")
                        nc.tensor.transpose(
                            pt[:], vT[:, kt * 128:(kt + 1) * 128], id_sb[:]
                        )
                        v3 = v_sb[:].rearrange("p (h e) -> p h e", h=HL)
                        nc.gpsimd.tensor_copy(v3[:, :, 64], onesc[:])
                        nc.vector.tensor_copy(
                            v3[:, :, 0:64],
                            pt[:].rearrange("p (h e) -> p h e", h=HL),
                        )
                        V[b][kt] = v_sb

            # ============ phase 2: attention, head-row outer ============
            for h in range(HL):
                h0 = h * 64
                ytn = [None] * (B * NQ)
                # denominator collector: rows 4m..4m+3 hold sums of (b,j)=m
                coll = wk.tile([B * NQ * 4, 128], F32, name="coll",
                               tag="coll", bufs=2)
                for b in range(B):
                    for j in range(NQ):
                        m = b * NQ + j
                        q0 = j * 512
                        py = ps_y.tile([65, 512], F32, name="p_y", tag="py")
                        n_kt = 4 * j + 4
                        for kt in range(n_kt):
                            i = kt - 4 * j  # diagonal index if >= 0
                            off = 128 * i if i >= 0 else 0
                            pss = ps_s.tile([128, 512], F32, name="p_s",
                                            tag="pss")
                            nc.tensor.matmul(
                                pss[:, off:512],
                                kT[b][h0:h0 + 64, kt * 128:(kt + 1) * 128],
                                qT[b][h0:h0 + 64, q0 + off:q0 + 512],
                                start=True,
                                stop=True,
                            )
                            p_sb = wk.tile([128, 512], F32R, name="p_sb",
                                           tag="p_sb", bufs=4)
                            nc.scalar.activation(
                                p_sb[:, off:512], pss[:, off:512], AF.Exp,
                                scale=float(SCALE),
                            )
                            if i >= 0:
                                nc.vector.tensor_mul(
                                    p_sb[:, off:off + 128],
                                    p_sb[:, off:off + 128],
                                    tri_sb[:],
                                )
                            if apply_pad_mask:
                                nc.vector.tensor_scalar_mul(
                                    p_sb[:, off:512],
                                    p_sb[:, off:512],
                                    padk_sb[:, b * NK + kt:b * NK + kt + 1],
                                )
                            nc.tensor.matmul(
                                py[0:65, off:512],
                                V[b][kt][:, h * 65:(h + 1) * 65],
                                p_sb[:, off:512],
                                start=(kt == 0),
                                stop=(kt == n_kt - 1),
                            )
                        # evacuate PV accumulator: unnormalized yT + sums
                        yu = ytp.tile([64, 512], F32R, name="ytn",
                                      tag=f"ytn{m}")
                        nc.vector.tensor_copy(yu[:], py[0:64, :])
                        ytn[m] = yu
                        srow = wk.tile([65, 512], F32, name="srow",
                                       tag="srow", bufs=4)
                        nc.vector.tensor_copy(srow[64:65, :], py[64:65, :])
                        # scatter the 512 sums over 4 partitions x 128
                        # (DMA pairs elements by iteration order)
                        nc.sync.dma_start(coll[4 * m:4 * m + 4, :],
                                          srow[64:65, :])
                # one wide reciprocal for all (b, j) of this head-row
                rcol = wk.tile([B * NQ * 4, 128], F32R, name="rcol",
                               tag="rcol", bufs=2)
                with nc.allow_low_precision(reason="fp32r softmax denom"):
                    nc.vector.reciprocal(rcol[:], coll[:])
                for m in range(B * NQ):
                    rbase = 32 * (m % 3)
                    rr = wk.tile([65, 512], F32R, name="rrow",
                                 tag=f"rr{m // 3}", bufs=2)
                    # gather 4x128 back into one 512-wide row at a
                    # 32-aligned partition (K=1 matmul tile_position rule)
                    nc.sync.dma_start(rr[rbase:rbase + 1, :],
                                      rcol[4 * m:4 * m + 4, :])
                    pb = ps_main.tile([64, 512], F32, name="p_b", tag="ps")
                    nc.tensor.matmul(
                        pb[:],
                        ones_sb[rbase:rbase + 1, :],
                        rr[rbase:rbase + 1, :],
                        start=True,
                        stop=True,
                    )
                    nc.vector.tensor_mul(ytn[m][:], ytn[m][:], pb[:])
                    nc.sync.dma_start(a2a_in[h][m, :, :], ytn[m][:])

                nc.gpsimd.collective_compute(
                    "AllToAll",
                    mybir.AluOpType.bypass,
                    replica_groups=[list(range(N_CORES))],
                    ins=[a2a_in[h].opt()],
                    outs=[a2a_out[h].opt()],
                )

            # ============ phase 3: output projection ============
            with (
                tc.tile_pool(name="wo_pool", bufs=1) as wop,
                tc.tile_pool(name="ytf_pool", bufs=1) as yfp,
            ):
                wo_sb = []
                for ct in range(NCT):
                    w_sb = wop.tile([128, C], F32R, name=f"wo{ct}",
                                    tag=f"wo{ct}")
                    nc.sync.dma_start(w_sb[:], wo[ct * 128:(ct + 1) * 128, :])
                    wo_sb.append(w_sb)
                ytf = []
                for s in range(N_CORES):
                    y_sb = yfp.tile([128, 512], F32R, name=f"ytf{s}",
                                    tag=f"ytf{s}")
                    for h in range(HL):
                        nc.sync.dma_start(
                            y_sb[h * 64:(h + 1) * 64, :], a2a_out[h][s, :, :]
                        )
                    ytf.append(y_sb)
                for mt in range(4):
                    o_sb = wk.tile([128, C], F32, name="o_sb", tag="o_sb")
                    for n in range(2):
                        po = ps_main.tile([128, 512], F32, name="p_o",
                                          tag="ps")
                        for ct in range(NCT):
                            nc.tensor.matmul(
                                po[:],
                                ytf[ct][:, mt * 128:(mt + 1) * 128],
                                wo_sb[ct][:, n * 512:(n + 1) * 512],
                                start=(ct == 0),
                                stop=(ct == NCT - 1),
                            )
                        nc.vector.tensor_copy(
                            o_sb[:, n * 512:(n + 1) * 512], po[:]
                        )
                    nc.sync.dma_start(out[mt * 128:(mt + 1) * 128, :],
                                      o_sb[:])

    nc.compile()
    return nc


def _host_inputs(x, tok_mask, Wqkv, Wproj, apply_pad_mask):
    x = np.ascontiguousarray(np.asarray(x, dtype=np.float32))
    Wqkv = np.ascontiguousarray(np.asarray(Wqkv, dtype=np.float32))
    Wproj = np.ascontiguousarray(np.asarray(Wproj, dtype=np.float32))
    # xT: [C, B*T], time-major = [b0 t0..T-1 | b1 t0..T-1], pre-rounded
    xT = round_f32r(np.concatenate([x[b].T for b in range(B)], axis=1))
    wo_r = round_f32r(Wproj)
    r = np.arange(128)
    tri = (r[None, :] >= r[:, None]).astype(np.float32)  # keep if col >= row
    ident = np.eye(128, dtype=np.float32)
    if apply_pad_mask:
        padk = np.zeros((128, B * NK), np.float32)
        for b in range(B):
            padk[:, b * NK:(b + 1) * NK] = (
                np.asarray(tok_mask[b]).reshape(NK, 128).T.astype(np.float32)
            )
    else:
        padk = np.ones((128, B * NK), np.float32)

    in_maps = []
    for core in range(N_CORES):
        cols = slice(core * HL * D, (core + 1) * HL * D)  # 128 columns
        wqkv_c = round_f32r(
            np.concatenate(
                [Wqkv[:, :C][:, cols], Wqkv[:, C:2 * C][:, cols],
                 Wqkv[:, 2 * C:][:, cols]],
                axis=1,
            )
        )
        in_maps.append(
            {
                "xT": xT,
                "wqkv": wqkv_c,
                "wo": wo_r,
                "tri": tri,
                "ident": ident,
                "padk": padk,
            }
        )
    return in_maps


def kernel(x, tok_mask, Wqkv, Wproj, _run_kwargs=None):
    tok = np.asarray(tok_mask)
    apply_pad_mask = not bool(tok.all())
    key = apply_pad_mask
    if key not in _BUILD_CACHE:
        _BUILD_CACHE[key] = build_kernel(apply_pad_mask)
    nc = _BUILD_CACHE[key]
    in_maps = _host_inputs(x, tok_mask, Wqkv, Wproj, apply_pad_mask)
    kw = dict(_run_kwargs or {})
    res = bass_utils.run_bass_kernel_spmd(
        nc, in_maps, core_ids=list(range(N_CORES)), **kw
    )
    out = np.empty((B, T, C), np.float32)
    for core in range(N_CORES):
        b, jj = divmod(core, NQ)
        out[b, jj * 512:(jj + 1) * 512, :] = res.results[core]["out"]
    kernel.last_result = res
    return out


# revision 10
# speedup vs baseline: 1.3328x; 1.1342x over previous
"""Multi-head causal self-attention on 8 Trainium2 NeuronCores (Bass/Tile).

Problem: y = proj(softmax(causal_mask(Q K^T / sqrt(D))) V) for B=2, T=2048,
C=1024, H=16 heads, D=64.

Sharding (tensor-parallel over heads, 8-way):
  - Core i owns heads {2i, 2i+1}: computes qT/kT/vT for its heads over both
    batches (full x, its 128-column slice of Wqkv) and runs causal attention
    per head fully on-core, producing normalized yT_local (head-dims on
    partitions, time on the free axis).
  - Two 8-way AllToAlls (one per local head-row; the first overlaps the
    second head's compute) reshard head-split -> time-split: core j ends
    with yT_full [1024, 512] for time-slice j of the flattened (B*T) axis
    and computes its [512, 1024] slice of y @ Wproj.
  - The host concatenates the 8 time-slices into [2, 2048, 1024].

Matmuls use the float32r PE path (fp32 rounded to 11 explicit mantissa
bits; measured 227 ns per 512-wide matmul back-to-back — same rate as
bf16 — with fp32 accumulation in PSUM). fp32r operands must carry the
float32r dtype from their producer: host tensors are pre-rounded and
DMA'd into float32r tiles; on-chip producers write float32r directly.

Attention is computed transposed (S^T[k, q], keys on partitions): no
transposes anywhere in the attention path, exp on ScalarE straight out of
PSUM, and the softmax denominator comes free from a ones column appended
to V (row 64 of the P@V accumulator). Causality is exact: S^T blocks
strictly above the diagonal are skipped, diagonal blocks use a restricted
column range plus a triangular multiplicative mask after exp. Full blocks
are computed in 1024-wide pairs so one ACTIVATE covers two blocks.

The kernel is emitted with interleaved instruction streams (generators):
batch-1 projections are woven into head-0 attention and the output
projection into head-1 attention, so the in-order PE queue always has
independent matmuls to run while ScalarE works through exp. Softmax
normalization is deferred per (head, batch): denominators are DMA-packed
into a [16, 128] tile, inverted with one wide reciprocal, and broadcast
across partitions with K=1 PE matmuls, keeping slow reciprocals off the
PE critical path.
"""

import numpy as np

import concourse.bass as bass
import concourse.mybir as mybir
import concourse.tile as tile
from concourse import bacc
from concourse import bass_utils

F32 = mybir.dt.float32
F32R = mybir.dt.float32r
AF = mybir.ActivationFunctionType

B, T, C = 2, 2048, 1024
H, D = 16, 64
N_CORES = 8
HL = H // N_CORES        # heads per core = 2
NCT = C // 128           # contraction tiles = 8
NQ = T // 512            # q tiles per batch = 4
NK = T // 128            # k tiles per batch = 16
SCALE = 1.0 / float(np.sqrt(D))  # 0.125

_BUILD_CACHE = {}


def round_f32r(x):
    """fp32 -> fp32r rounding (11 explicit mantissa bits, nearest-even)."""
    u = np.asarray(x, np.float32).view(np.uint32).astype(np.uint64)
    low = u & np.uint64(0xFFF)
    base = u & np.uint64(0xFFFFF000)
    lsb = (u >> np.uint64(12)) & np.uint64(1)
    round_up = (low > 0x800) | ((low == 0x800) & (lsb == 1))
    out = base + np.where(round_up, np.uint64(0x1000), np.uint64(0))
    return (out & np.uint64(0xFFFFFFFF)).astype(np.uint32).view(np.float32)


def _drain(*gens):
    """Round-robin the generators until all are exhausted."""
    active = list(gens)
    while active:
        nxt = []
        for g in active:
            try:
                next(g)
                nxt.append(g)
            except StopIteration:
                pass
        active = nxt


def build_kernel(apply_pad_mask: bool):
    nc = bacc.Bacc(
        "TRN2", target_bir_lowering=False, debug=False, num_devices=N_CORES
    )
    xT = nc.dram_tensor("xT", [C, B * T], F32R, kind="ExternalInput").ap()
    wqkv = nc.dram_tensor("wqkv", [C, 3 * HL * D], F32R, kind="ExternalInput").ap()
    wo = nc.dram_tensor("wo", [C, C], F32R, kind="ExternalInput").ap()
    tri = nc.dram_tensor("tri", [128, 128], F32, kind="ExternalInput").ap()
    ident = nc.dram_tensor("ident", [128, 128], F32R, kind="ExternalInput").ap()
    padk = nc.dram_tensor("padk", [128, B * NK], F32, kind="ExternalInput").ap()
    out = nc.dram_tensor("out", [512, C], F32, kind="ExternalOutput").ap()

    with tile.TileContext(nc) as tc:
        with (
            tc.tile_pool(name="const", bufs=1) as constp,
            tc.tile_pool(name="qk", bufs=1) as qkp,
            tc.tile_pool(name="vv", bufs=1) as vvp,
            tc.tile_pool(name="xw", bufs=1) as xwp,
            tc.tile_pool(name="work", bufs=2) as wk,
            tc.tile_pool(name="ytmp_pool", bufs=2) as ytp,
            tc.tile_pool(name="ps_ss", bufs=2, space="PSUM") as ps_ss,
            tc.tile_pool(name="ps_main", bufs=2, space="PSUM") as ps_main,
            tc.tile_pool(name="ps_y", bufs=2, space="PSUM") as ps_y,
            tc.tile_pool(name="dram", bufs=1, space="DRAM") as dram,
        ):
            # ---------------- constants ----------------
            tri_sb = constp.tile([128, 128], F32, name="tri_sb")
            nc.sync.dma_start(tri_sb[:], tri[:])
            id_sb = constp.tile([128, 128], F32R, name="id_sb")
            nc.sync.dma_start(id_sb[:], ident[:])
            ones_f = constp.tile([65, 64], F32, name="ones_f")
            ones_sb = constp.tile([65, 64], F32R, name="ones_sb")
            for r in (0, 32, 64):
                nc.vector.memset(ones_f[r:r + 1, :], 1.0)
                nc.gpsimd.tensor_copy(ones_sb[r:r + 1, :], ones_f[r:r + 1, :])
            onesc_f = constp.tile([128, HL], F32, name="onesc_f")
            nc.vector.memset(onesc_f[:], 1.0)
            onesc = constp.tile([128, HL], F32R, name="onesc")
            nc.gpsimd.tensor_copy(onesc[:], onesc_f[:])
            if apply_pad_mask:
                padk_sb = constp.tile([128, B * NK], F32, name="padk_sb")
                nc.sync.dma_start(padk_sb[:], padk[:])

            a2a_in = [dram.tile([N_CORES, 64, 512], F32R, name=f"a2a_in{h}")
                      for h in range(HL)]
            a2a_out = [dram.tile([N_CORES, 64, 512], F32R, name=f"a2a_out{h}")
                       for h in range(HL)]

            # weights first so the first matmul group is ready ASAP
            wqkv_sb = []
            for ct in range(NCT):
                w_sb = xwp.tile([128, 3 * HL * D], F32R, name=f"wqkv{ct}",
                                tag=f"wqkv{ct}")
                nc.sync.dma_start(w_sb[:], wqkv[ct * 128:(ct + 1) * 128, :])
                wqkv_sb.append(w_sb)

            qT = [None] * B
            kT = [None] * B
            V = [[None] * NK for _ in range(B)]
            ytn = [[None] * (B * NQ) for _ in range(HL)]

            def qkv_emit(b):
                """Projections for batch b: yields between schedulable
                chunks so the PE stream can interleave with attention."""
                xt_sb = []
                for half in range(2):
                    for ct in range(NCT):
                        if half == 0:
                            x_sb = xwp.tile([128, T], F32R, name=f"xt{ct}",
                                            tag=f"xt{ct}")
                            xt_sb.append(x_sb)
                        nc.sync.dma_start(
                            xt_sb[ct][:, half * 1024:(half + 1) * 1024],
                            xT[ct * 128:(ct + 1) * 128,
                               b * T + half * 1024:b * T + (half + 1) * 1024],
                        )
                qT[b] = qkp.tile([128, T], F32R, name="qT", tag=f"qT{b}")
                kT[b] = qkp.tile([128, T], F32R, name="kT", tag=f"kT{b}")
                vT = qkp.tile([128, T], F32R, name="vT", tag="vT")
                for which, dst in ((2, vT), (1, kT[b]), (0, qT[b])):
                    for n in range(NQ):
                        p = ps_main.tile([128, 512], F32, name="p_mm",
                                         tag="ps")
                        for ct in range(NCT):
                            nc.tensor.matmul(
                                p[:],
                                wqkv_sb[ct][:, which * 128:(which + 1) * 128],
                                xt_sb[ct][:, n * 512:(n + 1) * 512],
                                start=(ct == 0),
                                stop=(ct == NCT - 1),
                            )
                        nc.vector.tensor_copy(dst[:, n * 512:(n + 1) * 512],
                                              p[:])
                        yield
                for kt in range(NK):
                    v_sb = vvp.tile([128, HL * 65], F32R, name=f"V{b}_{kt}",
                                    tag=f"V{b}_{kt}")
                    pt = ps_main.tile([128, 128], F32R, name="p_tr", tag="ps")
                    nc.tensor.transpose(pt[:], vT[:, kt * 128:(kt + 1) * 128],
                                        id_sb[:])
                    v3 = v_sb[:].rearrange("p (h e) -> p h e", h=HL)
                    nc.gpsimd.tensor_copy(v3[:, :, 64], onesc[:])
                    nc.vector.tensor_copy(
                        v3[:, :, 0:64],
                        pt[:].rearrange("p (h e) -> p h e", h=HL),
                    )
                    V[b][kt] = v_sb
                    if kt % 4 == 3:
                        yield

            def attn_emit(h, b):
                """Attention for head-row h, batch b. Yields per exp-block."""
                h0 = h * 64
                coll = wk.tile([4 * NQ, 128], F32, name="coll",
                               tag="coll", bufs=3)
                for j in range(NQ):
                    q0 = j * 512
                    py = ps_y.tile([65, 512], F32, name="p_y", tag="py")
                    n_kt = 4 * j + 4
                    # paired full blocks, then restricted diagonal singles
                    chunks = []
                    kt = 0
                    while kt < 4 * j:
                        chunks.append((kt, kt + 1))
                        kt += 2
                    for kt in range(4 * j, n_kt):
                        chunks.append((kt,))
                    for chunk in chunks:
                        pss = ps_ss.tile([128, 1024], F32, name="p_s",
                                         tag="pss")
                        lo = None
                        for ci, kt in enumerate(chunk):
                            i = kt - 4 * j
                            off = 128 * i if i >= 0 else 0
                            base = 512 * ci
                            if lo is None:
                                lo = base + off
                            nc.tensor.matmul(
                                pss[:, base + off:base + 512],
                                kT[b][h0:h0 + 64, kt * 128:(kt + 1) * 128],
                                qT[b][h0:h0 + 64, q0 + off:q0 + 512],
                                start=True,
                                stop=True,
                            )
                        hi = 512 * (len(chunk) - 1) + 512
                        p_sb = wk.tile([128, 1024], F32R, name="p_sb",
                                       tag="p_sb", bufs=3)
                        nc.scalar.activation(
                            p_sb[:, lo:hi], pss[:, lo:hi], AF.Exp,
                            scale=float(SCALE),
                        )
                        for ci, kt in enumerate(chunk):
                            i = kt - 4 * j
                            off = 128 * i if i >= 0 else 0
                            base = 512 * ci
                            if i >= 0:
                                nc.vector.tensor_mul(
                                    p_sb[:, base + off:base + off + 128],
                                    p_sb[:, base + off:base + off + 128],
                                    tri_sb[:],
                                )
                            if apply_pad_mask:
                                nc.vector.tensor_scalar_mul(
                                    p_sb[:, base + off:base + 512],
                                    p_sb[:, base + off:base + 512],
                                    padk_sb[:, b * NK + kt:b * NK + kt + 1],
                                )
                            nc.tensor.matmul(
                                py[0:65, off:512],
                                V[b][kt][:, h * 65:(h + 1) * 65],
                                p_sb[:, base + off:base + 512],
                                start=(kt == 0),
                                stop=(kt == n_kt - 1),
                            )
                        yield
                    # evacuate PV accumulator
                    m = b * NQ + j
                    yu = ytp.tile([64, 512], F32R, name="ytn",
                                  tag=f"ytn{m}", bufs=1)
                    nc.vector.tensor_copy(yu[:], py[0:64, :])
                    ytn[h][m] = yu
                    srow = wk.tile([65, 512], F32, name="srow", tag="srow",
                                   bufs=4)
                    nc.vector.tensor_copy(srow[64:65, :], py[64:65, :])
                    nc.sync.dma_start(coll[4 * j:4 * j + 4, :],
                                      srow[64:65, :])
                # wide reciprocal for this (h, b): all lanes busy
                rcol = wk.tile([4 * NQ, 128], F32R, name="rcol", tag="rcol",
                               bufs=3)
                with nc.allow_low_precision(reason="fp32r softmax denom"):
                    nc.vector.reciprocal(rcol[:], coll[:])
                for j in range(NQ):
                    m = b * NQ + j
                    rbase = 32 * (j % 3)
                    rr = wk.tile([65, 512], F32R, name="rrow",
                                 tag=f"rr{j // 3}", bufs=2)
                    nc.sync.dma_start(rr[rbase:rbase + 1, :],
                                      rcol[4 * j:4 * j + 4, :])
                    pb = ps_main.tile([64, 512], F32, name="p_b", tag="ps")
                    nc.tensor.matmul(
                        pb[:], ones_sb[rbase:rbase + 1, :],
                        rr[rbase:rbase + 1, :], start=True, stop=True,
                    )
                    nc.vector.tensor_mul(ytn[h][m][:], ytn[h][m][:], pb[:])
                    nc.sync.dma_start(a2a_in[h][m, :, :], ytn[h][m][:])
                    yield

            wo_sb = []
            ytf = []

            def wo_ytf0_emit():
                # prefetch Wproj into the (now dead) x slots and pull the
                # h=0 halves of yT_full as soon as AllToAll #1 lands
                for ct in range(NCT):
                    w_sb = xwp.tile([128, C], F32R, name=f"wo{ct}",
                                    tag=f"xt{ct}")
                    nc.sync.dma_start(w_sb[:], wo[ct * 128:(ct + 1) * 128, :])
                    wo_sb.append(w_sb)
                    yield
                for s in range(N_CORES):
                    y_sb = xwp.tile([128, 512], F32R, name=f"ytf{s}",
                                    tag=f"wqkv{s}")
                    nc.sync.dma_start(y_sb[0:64, :], a2a_out[0][s, :, :])
                    ytf.append(y_sb)
                    yield

            def proj_emit():
                for s in range(N_CORES):
                    nc.sync.dma_start(ytf[s][64:128, :], a2a_out[1][s, :, :])
                yield
                for mt in range(4):
                    o_sb = wk.tile([128, C], F32, name="o_sb", tag="o_sb")
                    for n in range(2):
                        po = ps_main.tile([128, 512], F32, name="p_o",
                                          tag="ps")
                        for ct in range(NCT):
                            nc.tensor.matmul(
                                po[:],
                                ytf[ct][:, mt * 128:(mt + 1) * 128],
                                wo_sb[ct][:, n * 512:(n + 1) * 512],
                                start=(ct == 0),
                                stop=(ct == NCT - 1),
                            )
                        nc.vector.tensor_copy(o_sb[:, n * 512:(n + 1) * 512],
                                              po[:])
                        yield
                    nc.sync.dma_start(out[mt * 128:(mt + 1) * 128, :],
                                      o_sb[:])

            # ---------------- emission schedule ----------------
            _drain(qkv_emit(0))
            _drain(attn_emit(0, 0), qkv_emit(1))
            _drain(attn_emit(0, 1))
            nc.gpsimd.collective_compute(
                "AllToAll", mybir.AluOpType.bypass,
                replica_groups=[list(range(N_CORES))],
                ins=[a2a_in[0].opt()], outs=[a2a_out[0].opt()],
            )
            _drain(attn_emit(1, 0), wo_ytf0_emit())
            _drain(attn_emit(1, 1))
            nc.gpsimd.collective_compute(
                "AllToAll", mybir.AluOpType.bypass,
                replica_groups=[list(range(N_CORES))],
                ins=[a2a_in[1].opt()], outs=[a2a_out[1].opt()],
            )
            _drain(proj_emit())

    nc.compile()
    return nc


def _host_inputs(x, tok_mask, Wqkv, Wproj, apply_pad_mask):
    x = np.ascontiguousarray(np.asarray(x, dtype=np.float32))
    Wqkv = np.ascontiguousarray(np.asarray(Wqkv, dtype=np.float32))
    Wproj = np.ascontiguousarray(np.asarray(Wproj, dtype=np.float32))
    xT = round_f32r(np.concatenate([x[b].T for b in range(B)], axis=1))
    wo_r = round_f32r(Wproj)
    r = np.arange(128)
    tri = (r[None, :] >= r[:, None]).astype(np.float32)  # keep if col >= row
    ident = np.eye(128, dtype=np.float32)
    if apply_pad_mask:
        padk = np.zeros((128, B * NK), np.float32)
        for b in range(B):
            padk[:, b * NK:(b + 1) * NK] = (
                np.asarray(tok_mask[b]).reshape(NK, 128).T.astype(np.float32)
            )
    else:
        padk = np.ones((128, B * NK), np.float32)

    in_maps = []
    for core in range(N_CORES):
        cols = slice(core * HL * D, (core + 1) * HL * D)
        wqkv_c = round_f32r(
            np.concatenate(
                [Wqkv[:, :C][:, cols], Wqkv[:, C:2 * C][:, cols],
                 Wqkv[:, 2 * C:][:, cols]],
                axis=1,
            )
        )
        in_maps.append(
            {
                "xT": xT,
                "wqkv": wqkv_c,
                "wo": wo_r,
                "tri": tri,
                "ident": ident,
                "padk": padk,
            }
        )
    return in_maps


def kernel(x, tok_mask, Wqkv, Wproj, _run_kwargs=None):
    tok = np.asarray(tok_mask)
    apply_pad_mask = not bool(tok.all())
    key = apply_pad_mask
    if key not in _BUILD_CACHE:
        _BUILD_CACHE[key] = build_kernel(apply_pad_mask)
    nc = _BUILD_CACHE[key]
    in_maps = _host_inputs(x, tok_mask, Wqkv, Wproj, apply_pad_mask)
    kw = dict(_run_kwargs or {})
    res = bass_utils.run_bass_kernel_spmd(
        nc, in_maps, core_ids=list(range(N_CORES)), **kw
    )
    out = np.empty((B, T, C), np.float32)
    for core in range(N_CORES):
        b, jj = divmod(core, NQ)
        out[b, jj * 512:(jj + 1) * 512, :] = res.results[core]["out"]
    kernel.last_result = res
    return out
